# revision 20
# baseline (speedup 1.0000x reference)
"""Trainium2 Bass kernel for nn_ConvSPE (depthwise-conv SPE + per-channel contraction).

Math (reference): per bn=(b,nu) row and channel d:
    pe_k = noise / sqrt(num*d)                       (b*num, d, s+2k)
    pe_q = depthwise_valid_xcorr(pe_k, w)            k=200 taps, same filter per channel
    qhat[b,nu,t] = sum_d pe_q[bn,d,t]      * q[b,d,t]
    khat[b,nu,t] = sum_d pe_k[bn,d,t+k//2] * k[b,d,t]

Kernel strategy (8 NeuronCores, data-parallel over 128 bn rows; 8 row-PAIRS/core):
  * Transposed-conv orientation: stationary = x-window [sample, (r,d)],
    moving = Toeplitz W_s[sample, t'] -> PSUM holds pe_q^T [(r,d), t'].
    3 PSUM-accumulated matmuls per (pair, t-block), 128 cols each.
  * q-path: ACT drains PSUM->SBUF fp16; DVE multiplies by replicated q^T
    (fp16 2x); the d-reduction is a PE matmul with the products as the
    STATIONARY and a [128,2] row-selector as moving -> cost 2 cols.
  * k-path: DVE mul vs shifted/scaled keys (fp16 2x); reduce over d via a
    split tree: Pool takes L1 + tail reduce, DVE takes L2+L3.
"""

import math
import numpy as np

_CACHE = {}


def _ensure_paths():
    try:
        import concourse  # noqa: F401
    except ImportError:
        import sys

        for p in ("/opt/trn_rl_repo", "/root/.axon_site/_ro/trn_rl_repo"):
            if p not in sys.path:
                sys.path.insert(0, p)


N_CORES = 8
B, D, L, K, NUM = 4, 64, 4096, 200, 32
NW = 34  # x windows of 128 per pair tile (covers t+j up to 4351)
NT = 32  # output time blocks of 128
NK = 33  # khat product blocks (u = t + 100 spans [0, 4224))
PAIRS = 8  # row-pairs per core (16 rows)


def build_module():
    """Build + compile the per-core Bass module (identical SPMD program)."""
    _ensure_paths()
    from contextlib import ExitStack

    import concourse.bacc as bacc
    import concourse.mybir as mybir
    import concourse.tile as tile

    F16 = mybir.dt.float16
    F32 = mybir.dt.float32
    X = mybir.AxisListType.X

    nc = bacc.Bacc(
        "TRN2", target_bir_lowering=False, debug=False, num_devices=N_CORES
    )

    xf_d = nc.dram_tensor("xf", [PAIRS, 128, NW, 128], F16, kind="ExternalInput").ap()
    # packed consts: [3*128 Toeplitz cols | 2 selector cols]
    wq_d = nc.dram_tensor("wq", [128, 3 * 128 + 2], F16, kind="ExternalInput").ap()
    qt_d = nc.dram_tensor("qt", [128, NT, 128], F16, kind="ExternalInput").ap()
    kt_d = nc.dram_tensor("kt", [128, NK, D], F16, kind="ExternalInput").ap()
    qo_d = nc.dram_tensor("qo", [PAIRS, 128, NT, 2], F32, kind="ExternalOutput").ap()
    ko_d = nc.dram_tensor("ko", [PAIRS, 128, NK, 2], F16, kind="ExternalOutput").ap()

    with tile.TileContext(nc) as tc, ExitStack() as ctx:
        wp = ctx.enter_context(tc.tile_pool(name="const", bufs=1))
        xp = ctx.enter_context(tc.tile_pool(name="x", bufs=3))
        pp = ctx.enter_context(tc.tile_pool(name="psum", bufs=2, space="PSUM"))
        op_ps = ctx.enter_context(tc.tile_pool(name="opsum", bufs=2, space="PSUM"))
        cp = ctx.enter_context(tc.tile_pool(name="peq", bufs=2))
        qp = ctx.enter_context(tc.tile_pool(name="prodq", bufs=2))
        kp = ctx.enter_context(tc.tile_pool(name="prodk", bufs=2))
        tp = ctx.enter_context(tc.tile_pool(name="tree", bufs=2))
        oq = ctx.enter_context(tc.tile_pool(name="outq", bufs=2))
        ok = ctx.enter_context(tc.tile_pool(name="outk", bufs=2))

        # consts: Toeplitz weights + selector in one DMA (small, gates conv)
        wq_t = wp.tile([128, 3 * 128 + 2], F16, tag="wq")
        nc.sync.dma_start(wq_t[:], wq_d[:])
        wts = [wq_t[:, 128 * s : 128 * (s + 1)] for s in range(3)]
        sel_t = wq_t[:, 384:386]

        xts = {}

        def load_x(pr):
            """Two-chunk load so the conv can start on the first half."""
            if pr >= PAIRS:
                return
            xt = xp.tile([128, NW, 128], F16, tag="xt", name=f"xt_{pr}")
            nc.sync.dma_start(xt[:, 0:18, :], xf_d[pr, :, 0:18, :])
            nc.sync.dma_start(xt[:, 18:NW, :], xf_d[pr, :, 18:NW, :])
            xts[pr] = xt

        # fill order tuned for DVE saturation: conv can start on xt0-A;
        # kmul(0) on xt0-B+kt; kmul(1) chunks and qmul(0) chunks arrive just
        # in time behind them.
        xt0 = xp.tile([128, NW, 128], F16, tag="xt", name="xt_0")
        xt1 = xp.tile([128, NW, 128], F16, tag="xt", name="xt_1")
        kt_t = wp.tile([128, NK, D], F16, tag="kt")
        qt_t = wp.tile([128, NT, 128], F16, tag="qt")
        nc.sync.dma_start(xt0[:, 0:18, :], xf_d[0, :, 0:18, :])
        nc.sync.dma_start(kt_t[:], kt_d[:])
        nc.sync.dma_start(xt0[:, 18:NW, :], xf_d[0, :, 18:NW, :])
        nc.sync.dma_start(xt1[:, 0:18, :], xf_d[1, :, 0:18, :])
        nc.sync.dma_start(qt_t[:, 0:16, :], qt_d[:, 0:16, :])
        nc.sync.dma_start(xt1[:, 18:NW, :], xf_d[1, :, 18:NW, :])
        nc.sync.dma_start(qt_t[:, 16:NT, :], qt_d[:, 16:NT, :])
        xts[0] = xt0
        xts[1] = xt1

        peq_t, pq_t, pk_t, t1_t, t3_t = {}, {}, {}, {}, {}

        def conv_chunk(pr, ch):
            if not (0 <= pr < PAIRS) or pr not in xts:
                return
            xt = xts[pr]
            if ch == 0:
                peq_t[pr] = cp.tile([128, NT, 128], F16, tag="peq", name=f"peq_{pr}")
            ps = pp.tile([128, 8, 128], F32, tag="ps", name=f"ps_{pr}_{ch}")
            for t8 in range(8):
                tau = ch * 8 + t8
                # W_2 is zero for output cols < 57: emit it narrow, mid-group
                nc.tensor.matmul(
                    ps[:, t8, :], xt[:, tau, :], wts[0], start=True, stop=False
                )
                nc.tensor.matmul(
                    ps[:, t8, 57:128],
                    xt[:, tau + 2, :],
                    wq_t[:, 313:384],
                    start=False,
                    stop=False,
                    skip_group_check=True,
                )
                nc.tensor.matmul(
                    ps[:, t8, :], xt[:, tau + 1, :], wts[1], start=False, stop=True
                )
            nc.scalar.copy(peq_t[pr][:, ch * 8 : (ch + 1) * 8, :], ps[:])

        def kmul(pr):
            if not (0 <= pr < PAIRS):
                return
            xt = xts[pr]
            pk = kp.tile([128, NK, 2, D], F16, tag="pk", name=f"pk_{pr}")
            if pr <= 1:
                # window-chunked so DVE can start on the first x/keys chunks
                for r in range(2):
                    nc.vector.tensor_mul(
                        pk[:, 0:18, r, :],
                        xt[:, 0:18, D * r : D * (r + 1)],
                        kt_t[:, 0:18, :],
                    )
                for r in range(2):
                    nc.vector.tensor_mul(
                        pk[:, 18:NK, r, :],
                        xt[:, 18:NK, D * r : D * (r + 1)],
                        kt_t[:, 18:NK, :],
                    )
            else:
                nc.vector.tensor_mul(pk[:, :, 0, :], xt[:, 0:NK, 0:D], kt_t[:])
                nc.vector.tensor_mul(pk[:, :, 1, :], xt[:, 0:NK, D:128], kt_t[:])
            pk_t[pr] = pk

        def qmul(pr):
            if not (0 <= pr < PAIRS):
                return
            peq = peq_t.pop(pr)
            pq = qp.tile([128, NT, 128], F16, tag="pq", name=f"pq_{pr}")
            if pr == 0 or pr == PAIRS - 1:
                # halves: starts earlier at fill, drains faster at the tail
                h = NT // 2
                nc.vector.tensor_mul(pq[:, 0:h, :], peq[:, 0:h, :], qt_t[:, 0:h, :])
                nc.vector.tensor_mul(
                    pq[:, h:NT, :], peq[:, h:NT, :], qt_t[:, h:NT, :]
                )
            else:
                nc.vector.tensor_mul(pq[:], peq[:], qt_t[:])
            pq_t[pr] = pq

        def qreduce(pr):
            """PE d-reduce for pair pr, then drain + DMA out."""
            if not (0 <= pr < PAIRS):
                return
            pq = pq_t.pop(pr)
            po = op_ps.tile([128, NT, 2], F32, tag="po", name=f"po_{pr}")
            for t in range(NT):
                nc.tensor.matmul(
                    po[:, t, :], pq[:, t, :], sel_t, start=True, stop=True
                )
            qo_s = oq.tile([128, NT, 2], F32, tag="qo", name=f"qo_{pr}")
            nc.scalar.copy(qo_s[:], po[:])
            nc.sync.dma_start(qo_d[pr], qo_s[:])

        def tree_hi(pr, on_dve=False):
            """L1 (Pool unless on_dve) producing t1."""
            if not (0 <= pr < PAIRS):
                return
            pk = pk_t.pop(pr)
            eng = nc.vector if on_dve else nc.gpsimd
            t1 = tp.tile([128, NK, 2, 32], F16, tag="t1", name=f"t1_{pr}")
            eng.tensor_add(t1[:], pk[:, :, :, 0:32], pk[:, :, :, 32:64])
            t1_t[pr] = t1

        def tree_mid(pr):
            """L2+L3 on DVE producing t3."""
            if not (0 <= pr < PAIRS):
                return
            t1 = t1_t.pop(pr)
            t2 = tp.tile([128, NK, 2, 16], F16, tag="t2", name=f"t2_{pr}")
            nc.vector.tensor_add(t2[:], t1[:, :, :, 0:16], t1[:, :, :, 16:32])
            t3 = tp.tile([128, NK, 2, 8], F16, tag="t3", name=f"t3_{pr}")
            nc.vector.tensor_add(t3[:], t2[:, :, :, 0:8], t2[:, :, :, 8:16])
            t3_t[pr] = t3

        def tree_tail(pr, on_dve=False):
            """L4-L6 (Pool unless on_dve) -> ko DMA."""
            if not (0 <= pr < PAIRS):
                return
            t3 = t3_t.pop(pr)
            eng = nc.vector if on_dve else nc.gpsimd
            t4 = tp.tile([128, NK, 2, 4], F16, tag="t4", name=f"t4_{pr}")
            eng.tensor_add(t4[:], t3[:, :, :, 0:4], t3[:, :, :, 4:8])
            t5 = tp.tile([128, NK, 2, 2], F16, tag="t5", name=f"t5_{pr}")
            eng.tensor_add(t5[:], t4[:, :, :, 0:2], t4[:, :, :, 2:4])
            ko_s = ok.tile([128, NK, 2], F16, tag="ko", name=f"ko_{pr}")
            eng.tensor_add(ko_s[:], t5[:, :, :, 0], t5[:, :, :, 1])
            nc.sync.dma_start(ko_d[pr], ko_s[:])

        LAST = PAIRS - 1
        for i in range(PAIRS + 2):
            load_x(i + 2)
            kmul(i)
            conv_chunk(i, 0)
            conv_chunk(i, 1)
            qreduce(i - 2)
            conv_chunk(i, 2)
            conv_chunk(i, 3)
            qmul(i - 1)
            if i - 1 == LAST:
                # last pair's tree entirely on DVE (after qmul so the qo
                # output chain starts first) to shorten the tail
                tree_hi(LAST, on_dve=True)
                tree_mid(LAST)
                tree_tail(LAST, on_dve=True)
            else:
                tree_hi(i - 1)
                tree_mid(i - 1)
            if i - 2 != LAST:
                tree_tail(i - 2)
            if i - 1 == LAST:
                qreduce(LAST)
                break

    nc.compile()
    return nc


def _get_module():
    if "nc" not in _CACHE:
        _CACHE["nc"] = build_module()
    return _CACHE["nc"]


def make_in_maps(queries, keys, noise, conv_weight, num):
    """Host-side shard + re-layout (all cheap numpy ops)."""
    num = int(np.asarray(num))
    queries = np.asarray(queries, dtype=np.float32)
    keys = np.asarray(keys, dtype=np.float32)
    noise = np.asarray(noise, dtype=np.float32)
    w = np.asarray(conv_weight, dtype=np.float32)[0, 0, :]
    scale = 1.0 / math.sqrt(num * D)

    # Toeplitz weights (scale folded in): W_s[p, m] = w[p + 128s - m] * scale
    p = np.arange(128)[:, None]
    m = np.arange(128)[None, :]
    Wq = np.zeros((3, 128, 128), np.float32)
    for s in range(3):
        j = p + 128 * s - m
        mask = (j >= 0) & (j < K)
        Wq[s][mask] = w[j[mask]] * scale
    Wq16 = Wq.astype(np.float16)

    # row selector: sel[rd, r'] = (rd // 64 == r')
    sel = np.zeros((128, 2), np.float16)
    sel[0:D, 0] = 1.0
    sel[D:128, 1] = 1.0
    # packed [128, 3*128+2]: Toeplitz cols s-major then selector
    wq_pack = np.concatenate(
        [Wq16.transpose(1, 0, 2).reshape(128, 3 * 128), sel], axis=1
    )

    # xf[c][pair][p, n, (r,d)] = noise[16c + 2*pair + r, d, 128n + p]
    xf = (
        noise[:, :, : NW * 128]
        .reshape(N_CORES, PAIRS, 2, D, NW, 128)
        .transpose(0, 1, 5, 4, 2, 3)
        .reshape(N_CORES, PAIRS, 128, NW, 128)
        .astype(np.float16)
    )
    # qt[b][(r,d), tau, t'] = queries[b, d, 128 tau + t']  (replicated over r)
    qt_half = queries.reshape(B, D, NT, 128).astype(np.float16)
    qt = np.concatenate([qt_half, qt_half], axis=1)  # (B, 128, NT, 128)
    # kt[b][p, n, d] = keys[b, d, 128n + p - 100] * scale (zero out of range)
    kpad = np.zeros((B, D, NK * 128), np.float32)
    kpad[:, :, K // 2 : K // 2 + L] = keys * scale
    kt = kpad.reshape(B, D, NK, 128).transpose(0, 3, 2, 1).astype(np.float16)

    in_maps = []
    for c in range(N_CORES):
        b = c // 2
        in_maps.append(
            {
                "xf": np.ascontiguousarray(xf[c]),
                "wq": wq_pack,
                "qt": np.ascontiguousarray(qt[b]),
                "kt": np.ascontiguousarray(kt[b]),
            }
        )
    return in_maps


def assemble_outputs(results):
    qhat = np.empty((B * NUM, L), np.float32)
    khat = np.empty((B * NUM, L), np.float32)
    for c in range(N_CORES):
        qo = results[c]["qo"]  # (PAIRS, 128, NT, 2) f32, [pr, t', tau, r]
        ko = results[c]["ko"].astype(np.float32)  # (PAIRS, 128, NK, 2)
        qarr = qo.transpose(0, 3, 2, 1).reshape(16, NT * 128)
        karr = ko.transpose(0, 3, 2, 1).reshape(16, NK * 128)
        qhat[16 * c : 16 * (c + 1)] = qarr
        khat[16 * c : 16 * (c + 1)] = karr[:, K // 2 : K // 2 + L]
    return (
        qhat.reshape(B, NUM, L),
        khat.reshape(B, NUM, L),
    )


def kernel(queries, keys, noise, conv_weight, num):
    _ensure_paths()
    from concourse import bass_utils

    in_maps = make_in_maps(queries, keys, noise, conv_weight, num)
    nc = _get_module()
    res = bass_utils.run_bass_kernel_spmd(nc, in_maps, core_ids=list(range(N_CORES)))
    return assemble_outputs(res.results)


# revision 24
# speedup vs baseline: 1.0540x; 1.0540x over previous
"""Trainium2 Bass kernel for nn_ConvSPE (depthwise-conv SPE + per-channel contraction).

Math (reference): per bn=(b,nu) row and channel d:
    pe_k = noise / sqrt(num*d)                       (b*num, d, s+2k)
    pe_q = depthwise_valid_xcorr(pe_k, w)            k=200 taps, same filter per channel
    qhat[b,nu,t] = sum_d pe_q[bn,d,t]      * q[b,d,t]
    khat[b,nu,t] = sum_d pe_k[bn,d,t+k//2] * k[b,d,t]

Kernel strategy (8 NeuronCores, data-parallel over 128 bn rows; 8 row-PAIRS/core):
  * Transposed-conv orientation: stationary = x-window [sample, (r,d)],
    moving = Toeplitz W_s[sample, t'] -> PSUM holds pe_q^T [(r,d), t'].
    3 PSUM-accumulated matmuls per (pair, t-block), 128 cols each.
  * q-path: ACT drains PSUM->SBUF fp16; DVE multiplies by replicated q^T
    (fp16 2x); the d-reduction is a PE matmul with the products as the
    STATIONARY and a [128,2] row-selector as moving -> cost 2 cols.
  * k-path: DVE mul vs shifted/scaled keys (fp16 2x); reduce over d via a
    split tree: Pool takes L1 + tail reduce, DVE takes L2+L3.
"""

import math
import numpy as np

_CACHE = {}


def _ensure_paths():
    try:
        import concourse  # noqa: F401
    except ImportError:
        import sys

        for p in ("/opt/trn_rl_repo", "/root/.axon_site/_ro/trn_rl_repo"):
            if p not in sys.path:
                sys.path.insert(0, p)


N_CORES = 8
B, D, L, K, NUM = 4, 64, 4096, 200, 32
NW = 34  # x windows of 128 per pair tile (covers t+j up to 4351)
NT = 32  # output time blocks of 128
NK = 33  # khat product blocks (u = t + 100 spans [0, 4224))
PAIRS = 8  # row-pairs per core (16 rows)


DEFAULT_CFG = {
    "fill": "A",          # A: xt0,kt,xt1,qt sequential; B: interleaved chunks
    "kmul_split": (),     # pairs whose kmul is window-chunked
    "qmul_split": (7,),   # pairs whose qmul is split in halves
    "tail_break": True,   # emit qreduce(LAST) immediately at slot LAST+1
    "last_tree_dve": False,
    "last_tail_dve": True,
}


def build_module(cfg=None):
    """Build + compile the per-core Bass module (identical SPMD program)."""
    cfg = {**DEFAULT_CFG, **(cfg or {})}
    _ensure_paths()
    from contextlib import ExitStack

    import concourse.bacc as bacc
    import concourse.mybir as mybir
    import concourse.tile as tile

    F16 = mybir.dt.float16
    F32 = mybir.dt.float32
    X = mybir.AxisListType.X

    nc = bacc.Bacc(
        "TRN2", target_bir_lowering=False, debug=False, num_devices=N_CORES
    )

    xf_d = nc.dram_tensor("xf", [PAIRS, 128, NW, 128], F16, kind="ExternalInput").ap()
    # packed consts: [3*128 Toeplitz cols | 2 selector cols]
    wq_d = nc.dram_tensor("wq", [128, 3 * 128 + 2], F16, kind="ExternalInput").ap()
    qt_d = nc.dram_tensor("qt", [128, NT, 128], F16, kind="ExternalInput").ap()
    kt_d = nc.dram_tensor("kt", [128, NK, D], F16, kind="ExternalInput").ap()
    out_d = nc.dram_tensor(
        "out", [PAIRS, 128, NT + NK, 2], F16, kind="ExternalOutput"
    ).ap()

    with tile.TileContext(nc) as tc, ExitStack() as ctx:
        wp = ctx.enter_context(tc.tile_pool(name="const", bufs=1))
        xp = ctx.enter_context(tc.tile_pool(name="x", bufs=3))
        pp = ctx.enter_context(tc.tile_pool(name="psum", bufs=2, space="PSUM"))
        op_ps = ctx.enter_context(tc.tile_pool(name="opsum", bufs=2, space="PSUM"))
        cp = ctx.enter_context(tc.tile_pool(name="peq", bufs=2))
        qp = ctx.enter_context(tc.tile_pool(name="prodq", bufs=2))
        kp = ctx.enter_context(tc.tile_pool(name="prodk", bufs=2))
        tp = ctx.enter_context(tc.tile_pool(name="tree", bufs=2))
        oq = ctx.enter_context(tc.tile_pool(name="outq", bufs=2))

        # consts: Toeplitz weights + selector in one DMA (small, gates conv)
        wq_t = wp.tile([128, 3 * 128 + 2], F16, tag="wq")
        nc.sync.dma_start(wq_t[:], wq_d[:])
        wts = [wq_t[:, 128 * s : 128 * (s + 1)] for s in range(3)]
        sel_t = wq_t[:, 384:386]

        xts = {}

        def load_x(pr):
            """Two-chunk load so the conv can start on the first half."""
            if pr >= PAIRS:
                return
            xt = xp.tile([128, NW, 128], F16, tag="xt", name=f"xt_{pr}")
            nc.sync.dma_start(xt[:, 0:18, :], xf_d[pr, :, 0:18, :])
            nc.sync.dma_start(xt[:, 18:NW, :], xf_d[pr, :, 18:NW, :])
            xts[pr] = xt

        kt_t = wp.tile([128, NK, D], F16, tag="kt")
        qt_t = wp.tile([128, NT, 128], F16, tag="qt")
        if cfg["fill"] == "A":
            load_x(0)
            nc.sync.dma_start(kt_t[:], kt_d[:])
            load_x(1)
            nc.sync.dma_start(qt_t[:], qt_d[:])
        else:
            xt0 = xp.tile([128, NW, 128], F16, tag="xt", name="xt_0")
            xt1 = xp.tile([128, NW, 128], F16, tag="xt", name="xt_1")
            nc.sync.dma_start(xt0[:, 0:18, :], xf_d[0, :, 0:18, :])
            nc.sync.dma_start(kt_t[:], kt_d[:])
            nc.sync.dma_start(xt0[:, 18:NW, :], xf_d[0, :, 18:NW, :])
            nc.sync.dma_start(xt1[:, 0:18, :], xf_d[1, :, 0:18, :])
            nc.sync.dma_start(qt_t[:, 0:16, :], qt_d[:, 0:16, :])
            nc.sync.dma_start(xt1[:, 18:NW, :], xf_d[1, :, 18:NW, :])
            nc.sync.dma_start(qt_t[:, 16:NT, :], qt_d[:, 16:NT, :])
            xts[0] = xt0
            xts[1] = xt1

        peq_t, pq_t, pk_t, t1_t, t3_t, out_t = {}, {}, {}, {}, {}, {}

        def conv_chunk(pr, ch):
            if not (0 <= pr < PAIRS) or pr not in xts:
                return
            xt = xts[pr]
            if ch == 0:
                peq_t[pr] = cp.tile([128, NT, 128], F16, tag="peq", name=f"peq_{pr}")
            ps = pp.tile([128, 8, 128], F32, tag="ps", name=f"ps_{pr}_{ch}")
            for t8 in range(8):
                tau = ch * 8 + t8
                # W_2 is zero for output cols < 57: emit it narrow, mid-group
                nc.tensor.matmul(
                    ps[:, t8, :], xt[:, tau, :], wts[0], start=True, stop=False
                )
                nc.tensor.matmul(
                    ps[:, t8, 57:128],
                    xt[:, tau + 2, :],
                    wq_t[:, 313:384],
                    start=False,
                    stop=False,
                    skip_group_check=True,
                )
                nc.tensor.matmul(
                    ps[:, t8, :], xt[:, tau + 1, :], wts[1], start=False, stop=True
                )
            nc.scalar.copy(peq_t[pr][:, ch * 8 : (ch + 1) * 8, :], ps[:])

        def kmul(pr):
            if not (0 <= pr < PAIRS):
                return
            xt = xts[pr]
            pk = kp.tile([128, NK, 2, D], F16, tag="pk", name=f"pk_{pr}")
            if pr in cfg["kmul_split"]:
                # window-chunked so DVE can start on the first x/keys chunks
                for r in range(2):
                    nc.vector.tensor_mul(
                        pk[:, 0:18, r, :],
                        xt[:, 0:18, D * r : D * (r + 1)],
                        kt_t[:, 0:18, :],
                    )
                for r in range(2):
                    nc.vector.tensor_mul(
                        pk[:, 18:NK, r, :],
                        xt[:, 18:NK, D * r : D * (r + 1)],
                        kt_t[:, 18:NK, :],
                    )
            else:
                nc.vector.tensor_mul(pk[:, :, 0, :], xt[:, 0:NK, 0:D], kt_t[:])
                nc.vector.tensor_mul(pk[:, :, 1, :], xt[:, 0:NK, D:128], kt_t[:])
            pk_t[pr] = pk

        def qmul(pr):
            if not (0 <= pr < PAIRS):
                return
            peq = peq_t.pop(pr)
            pq = qp.tile([128, NT, 128], F16, tag="pq", name=f"pq_{pr}")
            if pr in cfg["qmul_split"]:
                # halves: starts earlier at fill, drains faster at the tail
                h = NT // 2
                nc.vector.tensor_mul(pq[:, 0:h, :], peq[:, 0:h, :], qt_t[:, 0:h, :])
                nc.vector.tensor_mul(
                    pq[:, h:NT, :], peq[:, h:NT, :], qt_t[:, h:NT, :]
                )
            else:
                nc.vector.tensor_mul(pq[:], peq[:], qt_t[:])
            pq_t[pr] = pq

        def qreduce(pr):
            """PE d-reduce for pair pr, then drain + DMA out."""
            if not (0 <= pr < PAIRS):
                return
            pq = pq_t.pop(pr)
            po = op_ps.tile([128, NT, 2], F32, tag="po", name=f"po_{pr}")
            for t in range(NT):
                nc.tensor.matmul(
                    po[:, t, :], pq[:, t, :], sel_t, start=True, stop=True
                )
            out_s = oq.tile([128, NT + NK, 2], F16, tag="out", name=f"out_{pr}")
            nc.scalar.copy(out_s[:, 0:NT, :], po[:])
            out_t[pr] = out_s

        def tree_hi(pr, on_dve=False):
            """L1 (Pool unless on_dve) producing t1."""
            if not (0 <= pr < PAIRS):
                return
            pk = pk_t.pop(pr)
            eng = nc.vector if on_dve else nc.gpsimd
            t1 = tp.tile([128, NK, 2, 32], F16, tag="t1", name=f"t1_{pr}")
            eng.tensor_add(t1[:], pk[:, :, :, 0:32], pk[:, :, :, 32:64])
            t1_t[pr] = t1

        def tree_mid(pr):
            """L2+L3 on DVE producing t3."""
            if not (0 <= pr < PAIRS):
                return
            t1 = t1_t.pop(pr)
            t2 = tp.tile([128, NK, 2, 16], F16, tag="t2", name=f"t2_{pr}")
            nc.vector.tensor_add(t2[:], t1[:, :, :, 0:16], t1[:, :, :, 16:32])
            t3 = tp.tile([128, NK, 2, 8], F16, tag="t3", name=f"t3_{pr}")
            nc.vector.tensor_add(t3[:], t2[:, :, :, 0:8], t2[:, :, :, 8:16])
            t3_t[pr] = t3

        def tree_tail(pr, on_dve=False):
            """L4-L6 (Pool unless on_dve) -> ko DMA."""
            if not (0 <= pr < PAIRS):
                return
            t3 = t3_t.pop(pr)
            eng = nc.vector if on_dve else nc.gpsimd
            t4 = tp.tile([128, NK, 2, 4], F16, tag="t4", name=f"t4_{pr}")
            eng.tensor_add(t4[:], t3[:, :, :, 0:4], t3[:, :, :, 4:8])
            t5 = tp.tile([128, NK, 2, 2], F16, tag="t5", name=f"t5_{pr}")
            eng.tensor_add(t5[:], t4[:, :, :, 0:2], t4[:, :, :, 2:4])
            out_s = out_t.pop(pr)
            eng.tensor_add(
                out_s[:, NT : NT + NK, :], t5[:, :, :, 0], t5[:, :, :, 1]
            )
            nc.sync.dma_start(out_d[pr], out_s[:])

        LAST = PAIRS - 1
        for i in range(PAIRS + 2):
            load_x(i + 2)
            kmul(i)
            conv_chunk(i, 0)
            conv_chunk(i, 1)
            qreduce(i - 2)
            conv_chunk(i, 2)
            conv_chunk(i, 3)
            qmul(i - 1)
            if i - 1 == LAST and cfg["last_tree_dve"]:
                # last pair's tree entirely on DVE (after qmul so the qo
                # output chain starts first) to shorten the tail
                tree_hi(LAST, on_dve=True)
                tree_mid(LAST)
                tree_tail(LAST, on_dve=True)
            else:
                tree_hi(i - 1)
                tree_mid(i - 1)
            if i - 2 != LAST or not cfg["last_tree_dve"]:
                tree_tail(
                    i - 2, on_dve=(i - 2 == LAST and cfg["last_tail_dve"])
                )
            if i - 1 == LAST and cfg["tail_break"]:
                qreduce(LAST)
                break

    nc.compile()
    return nc


def _get_module():
    if "nc" not in _CACHE:
        _CACHE["nc"] = build_module()
    return _CACHE["nc"]


def make_in_maps(queries, keys, noise, conv_weight, num):
    """Host-side shard + re-layout (all cheap numpy ops)."""
    num = int(np.asarray(num))
    queries = np.asarray(queries, dtype=np.float32)
    keys = np.asarray(keys, dtype=np.float32)
    noise = np.asarray(noise, dtype=np.float32)
    w = np.asarray(conv_weight, dtype=np.float32)[0, 0, :]
    scale = 1.0 / math.sqrt(num * D)

    # Toeplitz weights (scale folded in): W_s[p, m] = w[p + 128s - m] * scale
    p = np.arange(128)[:, None]
    m = np.arange(128)[None, :]
    Wq = np.zeros((3, 128, 128), np.float32)
    for s in range(3):
        j = p + 128 * s - m
        mask = (j >= 0) & (j < K)
        Wq[s][mask] = w[j[mask]] * scale
    Wq16 = Wq.astype(np.float16)

    # row selector: sel[rd, r'] = (rd // 64 == r')
    sel = np.zeros((128, 2), np.float16)
    sel[0:D, 0] = 1.0
    sel[D:128, 1] = 1.0
    # packed [128, 3*128+2]: Toeplitz cols s-major then selector
    wq_pack = np.concatenate(
        [Wq16.transpose(1, 0, 2).reshape(128, 3 * 128), sel], axis=1
    )

    # xf[c][pair][p, n, (r,d)] = noise[16c + 2*pair + r, d, 128n + p]
    xf = (
        noise[:, :, : NW * 128]
        .reshape(N_CORES, PAIRS, 2, D, NW, 128)
        .transpose(0, 1, 5, 4, 2, 3)
        .reshape(N_CORES, PAIRS, 128, NW, 128)
        .astype(np.float16)
    )
    # qt[b][(r,d), tau, t'] = queries[b, d, 128 tau + t']  (replicated over r)
    qt_half = queries.reshape(B, D, NT, 128).astype(np.float16)
    qt = np.concatenate([qt_half, qt_half], axis=1)  # (B, 128, NT, 128)
    # kt[b][p, n, d] = keys[b, d, 128n + p - 100] * scale (zero out of range)
    kpad = np.zeros((B, D, NK * 128), np.float32)
    kpad[:, :, K // 2 : K // 2 + L] = keys * scale
    kt = kpad.reshape(B, D, NK, 128).transpose(0, 3, 2, 1).astype(np.float16)

    in_maps = []
    for c in range(N_CORES):
        b = c // 2
        in_maps.append(
            {
                "xf": np.ascontiguousarray(xf[c]),
                "wq": wq_pack,
                "qt": np.ascontiguousarray(qt[b]),
                "kt": np.ascontiguousarray(kt[b]),
            }
        )
    return in_maps


def assemble_outputs(results):
    qhat = np.empty((B * NUM, L), np.float32)
    khat = np.empty((B * NUM, L), np.float32)
    for c in range(N_CORES):
        out = results[c]["out"].astype(np.float32)  # (PAIRS, 128, NT+NK, 2)
        qarr = out[:, :, 0:NT, :].transpose(0, 3, 2, 1).reshape(16, NT * 128)
        karr = (
            out[:, :, NT : NT + NK, :].transpose(0, 3, 2, 1).reshape(16, NK * 128)
        )
        qhat[16 * c : 16 * (c + 1)] = qarr
        khat[16 * c : 16 * (c + 1)] = karr[:, K // 2 : K // 2 + L]
    return (
        qhat.reshape(B, NUM, L),
        khat.reshape(B, NUM, L),
    )


def kernel(queries, keys, noise, conv_weight, num):
    _ensure_paths()
    from concourse import bass_utils

    in_maps = make_in_maps(queries, keys, noise, conv_weight, num)
    nc = _get_module()
    res = bass_utils.run_bass_kernel_spmd(nc, in_maps, core_ids=list(range(N_CORES)))
    return assemble_outputs(res.results)


# revision 31
# speedup vs baseline: 1.0575x; 1.0033x over previous
"""Trainium2 Bass kernel for nn_ConvSPE (depthwise-conv SPE + per-channel contraction).

Math (reference): per bn=(b,nu) row and channel d:
    pe_k = noise / sqrt(num*d)                       (b*num, d, s+2k)
    pe_q = depthwise_valid_xcorr(pe_k, w)            k=200 taps, same filter per channel
    qhat[b,nu,t] = sum_d pe_q[bn,d,t]      * q[b,d,t]
    khat[b,nu,t] = sum_d pe_k[bn,d,t+k//2] * k[b,d,t]

Kernel strategy (8 NeuronCores, data-parallel over 128 bn rows; 8 row-PAIRS/core):
  * Transposed-conv orientation: stationary = x-window [sample, (r,d)],
    moving = Toeplitz W_s[sample, t'] -> PSUM holds pe_q^T [(r,d), t'].
    3 PSUM-accumulated matmuls per (pair, t-block), 128 cols each.
  * q-path: ACT drains PSUM->SBUF fp16; DVE multiplies by replicated q^T
    (fp16 2x); the d-reduction is a PE matmul with the products as the
    STATIONARY and a [128,2] row-selector as moving -> cost 2 cols.
  * k-path: DVE mul vs shifted/scaled keys (fp16 2x); reduce over d via a
    split tree: Pool takes L1 + tail reduce, DVE takes L2+L3.
"""

import math
import numpy as np

_CACHE = {}


def _ensure_paths():
    try:
        import concourse  # noqa: F401
    except ImportError:
        import sys

        for p in ("/opt/trn_rl_repo", "/root/.axon_site/_ro/trn_rl_repo"):
            if p not in sys.path:
                sys.path.insert(0, p)


N_CORES = 8
B, D, L, K, NUM = 4, 64, 4096, 200, 32
NW = 34  # x windows of 128 per pair tile (covers t+j up to 4351)
NT = 32  # output time blocks of 128
NK = 33  # khat product blocks (u = t + 100 spans [0, 4224))
PAIRS = 8  # row-pairs per core (16 rows)


DEFAULT_CFG = {
    "fill": "A",          # A: xt0,kt,xt1,qt sequential; B: interleaved chunks
    "kmul_split": (),     # pairs whose kmul is window-chunked
    "qmul_split": (7,),   # pairs whose qmul is split in halves
    "tail_break": True,   # emit qreduce(LAST) immediately at slot LAST+1
    "last_tree_dve": False,
    "last_tail_dve": True,
    "drain_chunks": 4,    # conv/drain chunks per pair (divisor of 32)
    "warmup": 0,          # dummy PE matmuls after wq load (p-state pre-ramp)
    "xsplit": 18,         # first-chunk windows of each xt load
    "xbufs": 3,
    "bufs3": False,       # peq/pq/pk/tree pools with 3 buffers
    "kmul_pos": 0,        # 0: before conv; 1: after conv half 0
    "tail_dve_pairs": (), # extra pairs whose tree tail runs on DVE
    "kmul_bcast": False,  # single-instruction kmul via broadcast keys AP
}


def build_module(cfg=None):
    """Build + compile the per-core Bass module (identical SPMD program)."""
    cfg = {**DEFAULT_CFG, **(cfg or {})}
    _ensure_paths()
    from contextlib import ExitStack

    import concourse.bacc as bacc
    import concourse.mybir as mybir
    import concourse.tile as tile

    F16 = mybir.dt.float16
    F32 = mybir.dt.float32
    X = mybir.AxisListType.X

    nc = bacc.Bacc(
        "TRN2", target_bir_lowering=False, debug=False, num_devices=N_CORES
    )

    xf_d = nc.dram_tensor("xf", [PAIRS, 128, NW, 128], F16, kind="ExternalInput").ap()
    # packed consts: [3*128 Toeplitz cols | 2 selector cols]
    wq_d = nc.dram_tensor("wq", [128, 3 * 128 + 2], F16, kind="ExternalInput").ap()
    qt_d = nc.dram_tensor("qt", [128, NT, 128], F16, kind="ExternalInput").ap()
    kt_d = nc.dram_tensor("kt", [128, NK, D], F16, kind="ExternalInput").ap()
    out_d = nc.dram_tensor(
        "out", [PAIRS, 128, NT + NK, 2], F16, kind="ExternalOutput"
    ).ap()

    with tile.TileContext(nc) as tc, ExitStack() as ctx:
        wp = ctx.enter_context(tc.tile_pool(name="const", bufs=1))
        xp = ctx.enter_context(tc.tile_pool(name="x", bufs=cfg["xbufs"]))
        pp = ctx.enter_context(tc.tile_pool(name="psum", bufs=2, space="PSUM"))
        op_ps = ctx.enter_context(tc.tile_pool(name="opsum", bufs=2, space="PSUM"))
        nb = 3 if cfg["bufs3"] else 2
        cp = ctx.enter_context(tc.tile_pool(name="peq", bufs=nb))
        qp = ctx.enter_context(tc.tile_pool(name="prodq", bufs=nb))
        kp = ctx.enter_context(tc.tile_pool(name="prodk", bufs=nb))
        tp = ctx.enter_context(tc.tile_pool(name="tree", bufs=nb))
        oq = ctx.enter_context(tc.tile_pool(name="outq", bufs=2))

        # consts: Toeplitz weights + selector in one DMA (small, gates conv)
        wq_t = wp.tile([128, 3 * 128 + 2], F16, tag="wq")
        nc.sync.dma_start(wq_t[:], wq_d[:])
        wts = [wq_t[:, 128 * s : 128 * (s + 1)] for s in range(3)]
        sel_t = wq_t[:, 384:386]

        if cfg["warmup"]:
            # dummy matmuls on the weights tile: keeps PE busy through its
            # p-state ramp while the first x tiles stream in
            wu = op_ps.tile([128, 128], F32, tag="wu")
            for _ in range(cfg["warmup"]):
                nc.tensor.matmul(wu[:], wts[0], wts[1], start=True, stop=True)

        xts = {}

        XS = cfg["xsplit"]

        def load_x(pr):
            """Two-chunk load so the conv can start on the first half."""
            if pr >= PAIRS:
                return
            xt = xp.tile([128, NW, 128], F16, tag="xt", name=f"xt_{pr}")
            nc.sync.dma_start(xt[:, 0:XS, :], xf_d[pr, :, 0:XS, :])
            nc.sync.dma_start(xt[:, XS:NW, :], xf_d[pr, :, XS:NW, :])
            xts[pr] = xt

        kt_t = wp.tile([128, NK, D], F16, tag="kt")
        qt_t = wp.tile([128, NT, 128], F16, tag="qt")
        if cfg["fill"] == "A":
            load_x(0)
            nc.sync.dma_start(kt_t[:], kt_d[:])
            load_x(1)
            nc.sync.dma_start(qt_t[:], qt_d[:])
        elif cfg["fill"] == "C":
            xt0 = xp.tile([128, NW, 128], F16, tag="xt", name="xt_0")
            xt1 = xp.tile([128, NW, 128], F16, tag="xt", name="xt_1")
            nc.sync.dma_start(xt0[:, 0:XS, :], xf_d[0, :, 0:XS, :])
            nc.sync.dma_start(kt_t[:, 0:XS, :], kt_d[:, 0:XS, :])
            nc.sync.dma_start(xt0[:, XS:NW, :], xf_d[0, :, XS:NW, :])
            nc.sync.dma_start(kt_t[:, XS:NK, :], kt_d[:, XS:NK, :])
            nc.sync.dma_start(xt1[:, 0:XS, :], xf_d[1, :, 0:XS, :])
            nc.sync.dma_start(qt_t[:, 0:16, :], qt_d[:, 0:16, :])
            nc.sync.dma_start(xt1[:, XS:NW, :], xf_d[1, :, XS:NW, :])
            nc.sync.dma_start(qt_t[:, 16:NT, :], qt_d[:, 16:NT, :])
            xts[0] = xt0
            xts[1] = xt1
        else:
            xt0 = xp.tile([128, NW, 128], F16, tag="xt", name="xt_0")
            xt1 = xp.tile([128, NW, 128], F16, tag="xt", name="xt_1")
            nc.sync.dma_start(xt0[:, 0:18, :], xf_d[0, :, 0:18, :])
            nc.sync.dma_start(kt_t[:], kt_d[:])
            nc.sync.dma_start(xt0[:, 18:NW, :], xf_d[0, :, 18:NW, :])
            nc.sync.dma_start(xt1[:, 0:18, :], xf_d[1, :, 0:18, :])
            nc.sync.dma_start(qt_t[:, 0:16, :], qt_d[:, 0:16, :])
            nc.sync.dma_start(xt1[:, 18:NW, :], xf_d[1, :, 18:NW, :])
            nc.sync.dma_start(qt_t[:, 16:NT, :], qt_d[:, 16:NT, :])
            xts[0] = xt0
            xts[1] = xt1

        peq_t, pq_t, pk_t, t1_t, t3_t, out_t = {}, {}, {}, {}, {}, {}

        NCH = cfg["drain_chunks"]
        CW = NT // NCH

        def conv_chunk(pr, half):
            """Emit conv+drain for half in {0,1} as NCH/2 chunks."""
            for ch in range(half * NCH // 2, (half + 1) * NCH // 2):
                conv_chunk1(pr, ch)

        def conv_chunk1(pr, ch):
            if not (0 <= pr < PAIRS) or pr not in xts:
                return
            xt = xts[pr]
            if ch == 0:
                peq_t[pr] = cp.tile([128, NT, 128], F16, tag="peq", name=f"peq_{pr}")
            ps = pp.tile([128, CW, 128], F32, tag="ps", name=f"ps_{pr}_{ch}")
            for t8 in range(CW):
                tau = ch * CW + t8
                # W_2 is zero for output cols < 57: emit it narrow, mid-group
                nc.tensor.matmul(
                    ps[:, t8, :], xt[:, tau, :], wts[0], start=True, stop=False
                )
                nc.tensor.matmul(
                    ps[:, t8, 57:128],
                    xt[:, tau + 2, :],
                    wq_t[:, 313:384],
                    start=False,
                    stop=False,
                    skip_group_check=True,
                )
                nc.tensor.matmul(
                    ps[:, t8, :], xt[:, tau + 1, :], wts[1], start=False, stop=True
                )
            nc.scalar.copy(peq_t[pr][:, ch * CW : (ch + 1) * CW, :], ps[:])

        def kmul(pr):
            if not (0 <= pr < PAIRS):
                return
            xt = xts[pr]
            pk = kp.tile([128, NK, 2, D], F16, tag="pk", name=f"pk_{pr}")
            if pr in cfg["kmul_split"]:
                # window-chunked so DVE can start on the first x/keys chunks
                for r in range(2):
                    nc.vector.tensor_mul(
                        pk[:, 0:18, r, :],
                        xt[:, 0:18, D * r : D * (r + 1)],
                        kt_t[:, 0:18, :],
                    )
                for r in range(2):
                    nc.vector.tensor_mul(
                        pk[:, 18:NK, r, :],
                        xt[:, 18:NK, D * r : D * (r + 1)],
                        kt_t[:, 18:NK, :],
                    )
            elif cfg["kmul_bcast"]:
                xv = xt[:, 0:NK, :].rearrange("p n (r d) -> p n r d", r=2)
                ktb = kt_t[:].unsqueeze(2).broadcast_to((128, NK, 2, D))
                nc.vector.tensor_mul(pk[:], xv, ktb)
            else:
                nc.vector.tensor_mul(pk[:, :, 0, :], xt[:, 0:NK, 0:D], kt_t[:])
                nc.vector.tensor_mul(pk[:, :, 1, :], xt[:, 0:NK, D:128], kt_t[:])
            pk_t[pr] = pk

        def qmul(pr):
            if not (0 <= pr < PAIRS):
                return
            peq = peq_t.pop(pr)
            pq = qp.tile([128, NT, 128], F16, tag="pq", name=f"pq_{pr}")
            if pr in cfg["qmul_split"]:
                # halves: starts earlier at fill, drains faster at the tail
                h = NT // 2
                nc.vector.tensor_mul(pq[:, 0:h, :], peq[:, 0:h, :], qt_t[:, 0:h, :])
                nc.vector.tensor_mul(
                    pq[:, h:NT, :], peq[:, h:NT, :], qt_t[:, h:NT, :]
                )
            else:
                nc.vector.tensor_mul(pq[:], peq[:], qt_t[:])
            pq_t[pr] = pq

        def qreduce(pr):
            """PE d-reduce for pair pr, then drain + DMA out."""
            if not (0 <= pr < PAIRS):
                return
            pq = pq_t.pop(pr)
            po = op_ps.tile([128, NT, 2], F32, tag="po", name=f"po_{pr}")
            for t in range(NT):
                nc.tensor.matmul(
                    po[:, t, :], pq[:, t, :], sel_t, start=True, stop=True
                )
            out_s = oq.tile([128, NT + NK, 2], F16, tag="out", name=f"out_{pr}")
            nc.scalar.copy(out_s[:, 0:NT, :], po[:])
            out_t[pr] = out_s

        def tree_hi(pr, on_dve=False):
            """L1 (Pool unless on_dve) producing t1."""
            if not (0 <= pr < PAIRS):
                return
            pk = pk_t.pop(pr)
            eng = nc.vector if on_dve else nc.gpsimd
            t1 = tp.tile([128, NK, 2, 32], F16, tag="t1", name=f"t1_{pr}")
            eng.tensor_add(t1[:], pk[:, :, :, 0:32], pk[:, :, :, 32:64])
            t1_t[pr] = t1

        def tree_mid(pr):
            """L2+L3 on DVE producing t3."""
            if not (0 <= pr < PAIRS):
                return
            t1 = t1_t.pop(pr)
            t2 = tp.tile([128, NK, 2, 16], F16, tag="t2", name=f"t2_{pr}")
            nc.vector.tensor_add(t2[:], t1[:, :, :, 0:16], t1[:, :, :, 16:32])
            t3 = tp.tile([128, NK, 2, 8], F16, tag="t3", name=f"t3_{pr}")
            nc.vector.tensor_add(t3[:], t2[:, :, :, 0:8], t2[:, :, :, 8:16])
            t3_t[pr] = t3

        def tree_tail(pr, on_dve=False):
            """L4-L6 (Pool unless on_dve) -> ko DMA."""
            if not (0 <= pr < PAIRS):
                return
            t3 = t3_t.pop(pr)
            eng = nc.vector if on_dve else nc.gpsimd
            t4 = tp.tile([128, NK, 2, 4], F16, tag="t4", name=f"t4_{pr}")
            eng.tensor_add(t4[:], t3[:, :, :, 0:4], t3[:, :, :, 4:8])
            t5 = tp.tile([128, NK, 2, 2], F16, tag="t5", name=f"t5_{pr}")
            eng.tensor_add(t5[:], t4[:, :, :, 0:2], t4[:, :, :, 2:4])
            out_s = out_t.pop(pr)
            eng.tensor_add(
                out_s[:, NT : NT + NK, :], t5[:, :, :, 0], t5[:, :, :, 1]
            )
            nc.sync.dma_start(out_d[pr], out_s[:])

        LAST = PAIRS - 1
        for i in range(PAIRS + 2):
            load_x(i + 2)
            if cfg["kmul_pos"] == 0:
                kmul(i)
            conv_chunk(i, 0)
            if cfg["kmul_pos"] == 1:
                kmul(i)
            qreduce(i - 2)
            conv_chunk(i, 1)
            qmul(i - 1)
            if i - 1 == LAST and cfg["last_tree_dve"]:
                # last pair's tree entirely on DVE (after qmul so the qo
                # output chain starts first) to shorten the tail
                tree_hi(LAST, on_dve=True)
                tree_mid(LAST)
                tree_tail(LAST, on_dve=True)
            else:
                tree_hi(i - 1)
                tree_mid(i - 1)
            if i - 2 != LAST or not cfg["last_tree_dve"]:
                tree_tail(
                    i - 2,
                    on_dve=(i - 2 == LAST and cfg["last_tail_dve"])
                    or (i - 2) in cfg["tail_dve_pairs"],
                )
            if i - 1 == LAST and cfg["tail_break"]:
                qreduce(LAST)
                tree_tail(LAST, on_dve=cfg["last_tail_dve"])
                break

    nc.compile()
    return nc


def _get_module():
    if "nc" not in _CACHE:
        _CACHE["nc"] = build_module()
    return _CACHE["nc"]


def make_in_maps(queries, keys, noise, conv_weight, num):
    """Host-side shard + re-layout (all cheap numpy ops)."""
    num = int(np.asarray(num))
    queries = np.asarray(queries, dtype=np.float32)
    keys = np.asarray(keys, dtype=np.float32)
    noise = np.asarray(noise, dtype=np.float32)
    w = np.asarray(conv_weight, dtype=np.float32)[0, 0, :]
    scale = 1.0 / math.sqrt(num * D)

    # Toeplitz weights (scale folded in): W_s[p, m] = w[p + 128s - m] * scale
    p = np.arange(128)[:, None]
    m = np.arange(128)[None, :]
    Wq = np.zeros((3, 128, 128), np.float32)
    for s in range(3):
        j = p + 128 * s - m
        mask = (j >= 0) & (j < K)
        Wq[s][mask] = w[j[mask]] * scale
    Wq16 = Wq.astype(np.float16)

    # row selector: sel[rd, r'] = (rd // 64 == r')
    sel = np.zeros((128, 2), np.float16)
    sel[0:D, 0] = 1.0
    sel[D:128, 1] = 1.0
    # packed [128, 3*128+2]: Toeplitz cols s-major then selector
    wq_pack = np.concatenate(
        [Wq16.transpose(1, 0, 2).reshape(128, 3 * 128), sel], axis=1
    )

    # xf[c][pair][p, n, (r,d)] = noise[16c + 2*pair + r, d, 128n + p]
    xf = (
        noise[:, :, : NW * 128]
        .reshape(N_CORES, PAIRS, 2, D, NW, 128)
        .transpose(0, 1, 5, 4, 2, 3)
        .reshape(N_CORES, PAIRS, 128, NW, 128)
        .astype(np.float16)
    )
    # qt[b][(r,d), tau, t'] = queries[b, d, 128 tau + t']  (replicated over r)
    qt_half = queries.reshape(B, D, NT, 128).astype(np.float16)
    qt = np.concatenate([qt_half, qt_half], axis=1)  # (B, 128, NT, 128)
    # kt[b][p, n, d] = keys[b, d, 128n + p - 100] * scale (zero out of range)
    kpad = np.zeros((B, D, NK * 128), np.float32)
    kpad[:, :, K // 2 : K // 2 + L] = keys * scale
    kt = kpad.reshape(B, D, NK, 128).transpose(0, 3, 2, 1).astype(np.float16)

    in_maps = []
    for c in range(N_CORES):
        b = c // 2
        in_maps.append(
            {
                "xf": np.ascontiguousarray(xf[c]),
                "wq": wq_pack,
                "qt": np.ascontiguousarray(qt[b]),
                "kt": np.ascontiguousarray(kt[b]),
            }
        )
    return in_maps


def assemble_outputs(results):
    qhat = np.empty((B * NUM, L), np.float32)
    khat = np.empty((B * NUM, L), np.float32)
    for c in range(N_CORES):
        out = results[c]["out"].astype(np.float32)  # (PAIRS, 128, NT+NK, 2)
        qarr = out[:, :, 0:NT, :].transpose(0, 3, 2, 1).reshape(16, NT * 128)
        karr = (
            out[:, :, NT : NT + NK, :].transpose(0, 3, 2, 1).reshape(16, NK * 128)
        )
        qhat[16 * c : 16 * (c + 1)] = qarr
        khat[16 * c : 16 * (c + 1)] = karr[:, K // 2 : K // 2 + L]
    return (
        qhat.reshape(B, NUM, L),
        khat.reshape(B, NUM, L),
    )


def kernel(queries, keys, noise, conv_weight, num):
    _ensure_paths()
    from concourse import bass_utils

    in_maps = make_in_maps(queries, keys, noise, conv_weight, num)
    nc = _get_module()
    res = bass_utils.run_bass_kernel_spmd(nc, in_maps, core_ids=list(range(N_CORES)))
    return assemble_outputs(res.results)


# revision 33
# speedup vs baseline: 1.0584x; 1.0008x over previous
"""Trainium2 Bass kernel for nn_ConvSPE (depthwise-conv SPE + per-channel contraction).

Math (reference): per bn=(b,nu) row and channel d:
    pe_k = noise / sqrt(num*d)                       (b*num, d, s+2k)
    pe_q = depthwise_valid_xcorr(pe_k, w)            k=200 taps, same filter per channel
    qhat[b,nu,t] = sum_d pe_q[bn,d,t]      * q[b,d,t]
    khat[b,nu,t] = sum_d pe_k[bn,d,t+k//2] * k[b,d,t]

Kernel strategy (8 NeuronCores, data-parallel over 128 bn rows; 8 row-PAIRS/core):
  * Transposed-conv orientation: stationary = x-window [sample, (r,d)],
    moving = Toeplitz W_s[sample, t'] -> PSUM holds pe_q^T [(r,d), t'].
    3 PSUM-accumulated matmuls per (pair, t-block); the third Toeplitz
    band W_2 is zero for output cols < 57, so that matmul is emitted
    71 cols wide (saves ~15% PE time).
  * q-path: ACT drains PSUM->SBUF fp16; DVE multiplies by replicated q^T
    (fp16 2x); the d-reduction is a PE matmul with the products as the
    STATIONARY (Ldweights swaps are cheap) and a [128,2] row-selector as
    moving -> output free size 2, so the whole reduction is ~free on PE.
  * k-path: DVE mul vs shifted/scaled keys (fp16 2x); reduce over d via a
    split binary tree balanced across engines: Pool takes L1 (the big
    level) and L4-L6, DVE takes L2+L3.
  * Software pipeline over pairs (stage lag 1-2 slots) keeps DVE/Pool - the
    binding engines at ~43.5us busy each - gapless in steady state; qhat
    and khat are packed into one fp16 output tile per pair (single DMA).
"""

import math
import numpy as np

_CACHE = {}


def _ensure_paths():
    try:
        import concourse  # noqa: F401
    except ImportError:
        import sys

        for p in ("/opt/trn_rl_repo", "/root/.axon_site/_ro/trn_rl_repo"):
            if p not in sys.path:
                sys.path.insert(0, p)


N_CORES = 8
B, D, L, K, NUM = 4, 64, 4096, 200, 32
NW = 34  # x windows of 128 per pair tile (covers t+j up to 4351)
NT = 32  # output time blocks of 128
NK = 33  # khat product blocks (u = t + 100 spans [0, 4224))
PAIRS = 8  # row-pairs per core (16 rows)


DEFAULT_CFG = {
    "fill": "A",          # A: xt0,kt,xt1,qt sequential; B: interleaved chunks
    "kmul_split": (),     # pairs whose kmul is window-chunked
    "qmul_split": tuple(range(8)),  # pairs whose qmul is split in halves
    "tail_break": True,   # emit qreduce(LAST) immediately at slot LAST+1
    "last_tree_dve": False,
    "last_tail_dve": True,
    "drain_chunks": 4,    # conv/drain chunks per pair (divisor of 32)
    "warmup": 0,          # dummy PE matmuls after wq load (p-state pre-ramp)
    "xsplit": 18,         # first-chunk windows of each xt load
    "xbufs": 3,
    "bufs3": False,       # peq/pq/pk/tree pools with 3 buffers
    "kmul_pos": 0,        # 0: before conv; 1: after conv half 0
    "tail_dve_pairs": (), # extra pairs whose tree tail runs on DVE
    "kmul_bcast": False,  # single-instruction kmul via broadcast keys AP
}


def build_module(cfg=None):
    """Build + compile the per-core Bass module (identical SPMD program)."""
    cfg = {**DEFAULT_CFG, **(cfg or {})}
    _ensure_paths()
    from contextlib import ExitStack

    import concourse.bacc as bacc
    import concourse.mybir as mybir
    import concourse.tile as tile

    F16 = mybir.dt.float16
    F32 = mybir.dt.float32
    X = mybir.AxisListType.X

    nc = bacc.Bacc(
        "TRN2", target_bir_lowering=False, debug=False, num_devices=N_CORES
    )

    xf_d = nc.dram_tensor("xf", [PAIRS, 128, NW, 128], F16, kind="ExternalInput").ap()
    # packed consts: [3*128 Toeplitz cols | 2 selector cols]
    wq_d = nc.dram_tensor("wq", [128, 3 * 128 + 2], F16, kind="ExternalInput").ap()
    qt_d = nc.dram_tensor("qt", [128, NT, 128], F16, kind="ExternalInput").ap()
    kt_d = nc.dram_tensor("kt", [128, NK, D], F16, kind="ExternalInput").ap()
    out_d = nc.dram_tensor(
        "out", [PAIRS, 128, NT + NK, 2], F16, kind="ExternalOutput"
    ).ap()

    with tile.TileContext(nc) as tc, ExitStack() as ctx:
        wp = ctx.enter_context(tc.tile_pool(name="const", bufs=1))
        xp = ctx.enter_context(tc.tile_pool(name="x", bufs=cfg["xbufs"]))
        pp = ctx.enter_context(tc.tile_pool(name="psum", bufs=2, space="PSUM"))
        op_ps = ctx.enter_context(tc.tile_pool(name="opsum", bufs=2, space="PSUM"))
        nb = 3 if cfg["bufs3"] else 2
        cp = ctx.enter_context(tc.tile_pool(name="peq", bufs=nb))
        qp = ctx.enter_context(tc.tile_pool(name="prodq", bufs=nb))
        kp = ctx.enter_context(tc.tile_pool(name="prodk", bufs=nb))
        tp = ctx.enter_context(tc.tile_pool(name="tree", bufs=nb))
        oq = ctx.enter_context(tc.tile_pool(name="outq", bufs=2))

        # consts: Toeplitz weights + selector in one DMA (small, gates conv)
        wq_t = wp.tile([128, 3 * 128 + 2], F16, tag="wq")
        nc.sync.dma_start(wq_t[:], wq_d[:])
        wts = [wq_t[:, 128 * s : 128 * (s + 1)] for s in range(3)]
        sel_t = wq_t[:, 384:386]

        if cfg["warmup"]:
            # dummy matmuls on the weights tile: keeps PE busy through its
            # p-state ramp while the first x tiles stream in
            wu = op_ps.tile([128, 128], F32, tag="wu")
            for _ in range(cfg["warmup"]):
                nc.tensor.matmul(wu[:], wts[0], wts[1], start=True, stop=True)

        xts = {}

        XS = cfg["xsplit"]

        def load_x(pr):
            """Two-chunk load so the conv can start on the first half."""
            if pr >= PAIRS:
                return
            xt = xp.tile([128, NW, 128], F16, tag="xt", name=f"xt_{pr}")
            nc.sync.dma_start(xt[:, 0:XS, :], xf_d[pr, :, 0:XS, :])
            nc.sync.dma_start(xt[:, XS:NW, :], xf_d[pr, :, XS:NW, :])
            xts[pr] = xt

        kt_t = wp.tile([128, NK, D], F16, tag="kt")
        qt_t = wp.tile([128, NT, 128], F16, tag="qt")
        if cfg["fill"] == "A":
            load_x(0)
            nc.sync.dma_start(kt_t[:], kt_d[:])
            load_x(1)
            nc.sync.dma_start(qt_t[:], qt_d[:])
        elif cfg["fill"] == "C":
            xt0 = xp.tile([128, NW, 128], F16, tag="xt", name="xt_0")
            xt1 = xp.tile([128, NW, 128], F16, tag="xt", name="xt_1")
            nc.sync.dma_start(xt0[:, 0:XS, :], xf_d[0, :, 0:XS, :])
            nc.sync.dma_start(kt_t[:, 0:XS, :], kt_d[:, 0:XS, :])
            nc.sync.dma_start(xt0[:, XS:NW, :], xf_d[0, :, XS:NW, :])
            nc.sync.dma_start(kt_t[:, XS:NK, :], kt_d[:, XS:NK, :])
            nc.sync.dma_start(xt1[:, 0:XS, :], xf_d[1, :, 0:XS, :])
            nc.sync.dma_start(qt_t[:, 0:16, :], qt_d[:, 0:16, :])
            nc.sync.dma_start(xt1[:, XS:NW, :], xf_d[1, :, XS:NW, :])
            nc.sync.dma_start(qt_t[:, 16:NT, :], qt_d[:, 16:NT, :])
            xts[0] = xt0
            xts[1] = xt1
        else:
            xt0 = xp.tile([128, NW, 128], F16, tag="xt", name="xt_0")
            xt1 = xp.tile([128, NW, 128], F16, tag="xt", name="xt_1")
            nc.sync.dma_start(xt0[:, 0:18, :], xf_d[0, :, 0:18, :])
            nc.sync.dma_start(kt_t[:], kt_d[:])
            nc.sync.dma_start(xt0[:, 18:NW, :], xf_d[0, :, 18:NW, :])
            nc.sync.dma_start(xt1[:, 0:18, :], xf_d[1, :, 0:18, :])
            nc.sync.dma_start(qt_t[:, 0:16, :], qt_d[:, 0:16, :])
            nc.sync.dma_start(xt1[:, 18:NW, :], xf_d[1, :, 18:NW, :])
            nc.sync.dma_start(qt_t[:, 16:NT, :], qt_d[:, 16:NT, :])
            xts[0] = xt0
            xts[1] = xt1

        peq_t, pq_t, pk_t, t1_t, t3_t, out_t = {}, {}, {}, {}, {}, {}

        NCH = cfg["drain_chunks"]
        CW = NT // NCH

        def conv_chunk(pr, half):
            """Emit conv+drain for half in {0,1} as NCH/2 chunks."""
            for ch in range(half * NCH // 2, (half + 1) * NCH // 2):
                conv_chunk1(pr, ch)

        def conv_chunk1(pr, ch):
            if not (0 <= pr < PAIRS) or pr not in xts:
                return
            xt = xts[pr]
            if ch == 0:
                peq_t[pr] = cp.tile([128, NT, 128], F16, tag="peq", name=f"peq_{pr}")
            ps = pp.tile([128, CW, 128], F32, tag="ps", name=f"ps_{pr}_{ch}")
            for t8 in range(CW):
                tau = ch * CW + t8
                # W_2 is zero for output cols < 57: emit it narrow, mid-group
                nc.tensor.matmul(
                    ps[:, t8, :], xt[:, tau, :], wts[0], start=True, stop=False
                )
                nc.tensor.matmul(
                    ps[:, t8, 57:128],
                    xt[:, tau + 2, :],
                    wq_t[:, 313:384],
                    start=False,
                    stop=False,
                    skip_group_check=True,
                )
                nc.tensor.matmul(
                    ps[:, t8, :], xt[:, tau + 1, :], wts[1], start=False, stop=True
                )
            nc.scalar.copy(peq_t[pr][:, ch * CW : (ch + 1) * CW, :], ps[:])

        def kmul(pr):
            if not (0 <= pr < PAIRS):
                return
            xt = xts[pr]
            pk = kp.tile([128, NK, 2, D], F16, tag="pk", name=f"pk_{pr}")
            if pr in cfg["kmul_split"]:
                # window-chunked so DVE can start on the first x/keys chunks
                for r in range(2):
                    nc.vector.tensor_mul(
                        pk[:, 0:18, r, :],
                        xt[:, 0:18, D * r : D * (r + 1)],
                        kt_t[:, 0:18, :],
                    )
                for r in range(2):
                    nc.vector.tensor_mul(
                        pk[:, 18:NK, r, :],
                        xt[:, 18:NK, D * r : D * (r + 1)],
                        kt_t[:, 18:NK, :],
                    )
            elif cfg["kmul_bcast"]:
                xv = xt[:, 0:NK, :].rearrange("p n (r d) -> p n r d", r=2)
                ktb = kt_t[:].unsqueeze(2).broadcast_to((128, NK, 2, D))
                nc.vector.tensor_mul(pk[:], xv, ktb)
            else:
                nc.vector.tensor_mul(pk[:, :, 0, :], xt[:, 0:NK, 0:D], kt_t[:])
                nc.vector.tensor_mul(pk[:, :, 1, :], xt[:, 0:NK, D:128], kt_t[:])
            pk_t[pr] = pk

        def qmul(pr):
            if not (0 <= pr < PAIRS):
                return
            peq = peq_t.pop(pr)
            pq = qp.tile([128, NT, 128], F16, tag="pq", name=f"pq_{pr}")
            if pr in cfg["qmul_split"]:
                # halves: starts earlier at fill, drains faster at the tail
                h = NT // 2
                nc.vector.tensor_mul(pq[:, 0:h, :], peq[:, 0:h, :], qt_t[:, 0:h, :])
                nc.vector.tensor_mul(
                    pq[:, h:NT, :], peq[:, h:NT, :], qt_t[:, h:NT, :]
                )
            else:
                nc.vector.tensor_mul(pq[:], peq[:], qt_t[:])
            pq_t[pr] = pq

        def qreduce(pr):
            """PE d-reduce for pair pr, then drain + DMA out."""
            if not (0 <= pr < PAIRS):
                return
            pq = pq_t.pop(pr)
            po = op_ps.tile([128, NT, 2], F32, tag="po", name=f"po_{pr}")
            for t in range(NT):
                nc.tensor.matmul(
                    po[:, t, :], pq[:, t, :], sel_t, start=True, stop=True
                )
            out_s = oq.tile([128, NT + NK, 2], F16, tag="out", name=f"out_{pr}")
            nc.scalar.copy(out_s[:, 0:NT, :], po[:])
            out_t[pr] = out_s

        def tree_hi(pr, on_dve=False):
            """L1 (Pool unless on_dve) producing t1."""
            if not (0 <= pr < PAIRS):
                return
            pk = pk_t.pop(pr)
            eng = nc.vector if on_dve else nc.gpsimd
            t1 = tp.tile([128, NK, 2, 32], F16, tag="t1", name=f"t1_{pr}")
            eng.tensor_add(t1[:], pk[:, :, :, 0:32], pk[:, :, :, 32:64])
            t1_t[pr] = t1

        def tree_mid(pr):
            """L2+L3 on DVE producing t3."""
            if not (0 <= pr < PAIRS):
                return
            t1 = t1_t.pop(pr)
            t2 = tp.tile([128, NK, 2, 16], F16, tag="t2", name=f"t2_{pr}")
            nc.vector.tensor_add(t2[:], t1[:, :, :, 0:16], t1[:, :, :, 16:32])
            t3 = tp.tile([128, NK, 2, 8], F16, tag="t3", name=f"t3_{pr}")
            nc.vector.tensor_add(t3[:], t2[:, :, :, 0:8], t2[:, :, :, 8:16])
            t3_t[pr] = t3

        def tree_tail(pr, on_dve=False):
            """L4-L6 (Pool unless on_dve) -> ko DMA."""
            if not (0 <= pr < PAIRS):
                return
            t3 = t3_t.pop(pr)
            eng = nc.vector if on_dve else nc.gpsimd
            t4 = tp.tile([128, NK, 2, 4], F16, tag="t4", name=f"t4_{pr}")
            eng.tensor_add(t4[:], t3[:, :, :, 0:4], t3[:, :, :, 4:8])
            t5 = tp.tile([128, NK, 2, 2], F16, tag="t5", name=f"t5_{pr}")
            eng.tensor_add(t5[:], t4[:, :, :, 0:2], t4[:, :, :, 2:4])
            out_s = out_t.pop(pr)
            eng.tensor_add(
                out_s[:, NT : NT + NK, :], t5[:, :, :, 0], t5[:, :, :, 1]
            )
            nc.sync.dma_start(out_d[pr], out_s[:])

        LAST = PAIRS - 1
        for i in range(PAIRS + 2):
            load_x(i + 2)
            if cfg["kmul_pos"] == 0:
                kmul(i)
            conv_chunk(i, 0)
            if cfg["kmul_pos"] == 1:
                kmul(i)
            qreduce(i - 2)
            conv_chunk(i, 1)
            qmul(i - 1)
            if i - 1 == LAST and cfg["last_tree_dve"]:
                # last pair's tree entirely on DVE (after qmul so the qo
                # output chain starts first) to shorten the tail
                tree_hi(LAST, on_dve=True)
                tree_mid(LAST)
                tree_tail(LAST, on_dve=True)
            else:
                tree_hi(i - 1)
                tree_mid(i - 1)
            if i - 2 != LAST or not cfg["last_tree_dve"]:
                tree_tail(
                    i - 2,
                    on_dve=(i - 2 == LAST and cfg["last_tail_dve"])
                    or (i - 2) in cfg["tail_dve_pairs"],
                )
            if i - 1 == LAST and cfg["tail_break"]:
                qreduce(LAST)
                tree_tail(LAST, on_dve=cfg["last_tail_dve"])
                break

    nc.compile()
    return nc


def _get_module():
    if "nc" not in _CACHE:
        _CACHE["nc"] = build_module()
    return _CACHE["nc"]


def make_in_maps(queries, keys, noise, conv_weight, num):
    """Host-side shard + re-layout (all cheap numpy ops)."""
    num = int(np.asarray(num))
    queries = np.asarray(queries, dtype=np.float32)
    keys = np.asarray(keys, dtype=np.float32)
    noise = np.asarray(noise, dtype=np.float32)
    w = np.asarray(conv_weight, dtype=np.float32)[0, 0, :]
    scale = 1.0 / math.sqrt(num * D)

    # Toeplitz weights (scale folded in): W_s[p, m] = w[p + 128s - m] * scale
    p = np.arange(128)[:, None]
    m = np.arange(128)[None, :]
    Wq = np.zeros((3, 128, 128), np.float32)
    for s in range(3):
        j = p + 128 * s - m
        mask = (j >= 0) & (j < K)
        Wq[s][mask] = w[j[mask]] * scale
    Wq16 = Wq.astype(np.float16)

    # row selector: sel[rd, r'] = (rd // 64 == r')
    sel = np.zeros((128, 2), np.float16)
    sel[0:D, 0] = 1.0
    sel[D:128, 1] = 1.0
    # packed [128, 3*128+2]: Toeplitz cols s-major then selector
    wq_pack = np.concatenate(
        [Wq16.transpose(1, 0, 2).reshape(128, 3 * 128), sel], axis=1
    )

    # xf[c][pair][p, n, (r,d)] = noise[16c + 2*pair + r, d, 128n + p]
    xf = (
        noise[:, :, : NW * 128]
        .reshape(N_CORES, PAIRS, 2, D, NW, 128)
        .transpose(0, 1, 5, 4, 2, 3)
        .reshape(N_CORES, PAIRS, 128, NW, 128)
        .astype(np.float16)
    )
    # qt[b][(r,d), tau, t'] = queries[b, d, 128 tau + t']  (replicated over r)
    qt_half = queries.reshape(B, D, NT, 128).astype(np.float16)
    qt = np.concatenate([qt_half, qt_half], axis=1)  # (B, 128, NT, 128)
    # kt[b][p, n, d] = keys[b, d, 128n + p - 100] * scale (zero out of range)
    kpad = np.zeros((B, D, NK * 128), np.float32)
    kpad[:, :, K // 2 : K // 2 + L] = keys * scale
    kt = kpad.reshape(B, D, NK, 128).transpose(0, 3, 2, 1).astype(np.float16)

    in_maps = []
    for c in range(N_CORES):
        b = c // 2
        in_maps.append(
            {
                "xf": np.ascontiguousarray(xf[c]),
                "wq": wq_pack,
                "qt": np.ascontiguousarray(qt[b]),
                "kt": np.ascontiguousarray(kt[b]),
            }
        )
    return in_maps


def assemble_outputs(results):
    qhat = np.empty((B * NUM, L), np.float32)
    khat = np.empty((B * NUM, L), np.float32)
    for c in range(N_CORES):
        out = results[c]["out"].astype(np.float32)  # (PAIRS, 128, NT+NK, 2)
        qarr = out[:, :, 0:NT, :].transpose(0, 3, 2, 1).reshape(16, NT * 128)
        karr = (
            out[:, :, NT : NT + NK, :].transpose(0, 3, 2, 1).reshape(16, NK * 128)
        )
        qhat[16 * c : 16 * (c + 1)] = qarr
        khat[16 * c : 16 * (c + 1)] = karr[:, K // 2 : K // 2 + L]
    return (
        qhat.reshape(B, NUM, L),
        khat.reshape(B, NUM, L),
    )


def kernel(queries, keys, noise, conv_weight, num):
    _ensure_paths()
    from concourse import bass_utils

    in_maps = make_in_maps(queries, keys, noise, conv_weight, num)
    nc = _get_module()
    res = bass_utils.run_bass_kernel_spmd(nc, in_maps, core_ids=list(range(N_CORES)))
    return assemble_outputs(res.results)


# revision 36
# speedup vs baseline: 1.0791x; 1.0195x over previous
"""Trainium2 Bass kernel for nn_ConvSPE (depthwise-conv SPE + per-channel contraction).

Math (reference): per bn=(b,nu) row and channel d:
    pe_k = noise / sqrt(num*d)                       (b*num, d, s+2k)
    pe_q = depthwise_valid_xcorr(pe_k, w)            k=200 taps, same filter per channel
    qhat[b,nu,t] = sum_d pe_q[bn,d,t]      * q[b,d,t]
    khat[b,nu,t] = sum_d pe_k[bn,d,t+k//2] * k[b,d,t]

Kernel strategy (8 NeuronCores, data-parallel over 128 bn rows; 8 row-PAIRS/core):
  * Transposed-conv orientation: stationary = x-window [sample, (r,d)],
    moving = Toeplitz W_s[sample, t'] -> PSUM holds pe_q^T [(r,d), t'].
    3 PSUM-accumulated matmuls per (pair, t-block); the third Toeplitz
    band W_2 is zero for output cols < 57, so that matmul is emitted
    71 cols wide (saves ~15% PE time).
  * q-path: ACT drains PSUM->SBUF fp16; DVE multiplies by replicated q^T
    (fp16 2x); the d-reduction is a PE matmul with the products as the
    STATIONARY (Ldweights swaps are cheap) and a [128,2] row-selector as
    moving -> output free size 2, so the whole reduction is ~free on PE.
  * k-path: DVE mul vs shifted/scaled keys (fp16 2x); reduce over d via a
    split binary tree balanced across engines: Pool takes L1 (the big
    level) and L4-L6, DVE takes L2+L3.
  * Software pipeline over pairs (stage lag 1-2 slots) keeps DVE/Pool - the
    binding engines at ~43.5us busy each - gapless in steady state; qhat
    and khat are packed into one fp16 output tile per pair (single DMA).
"""

import math
import numpy as np

_CACHE = {}


def _ensure_paths():
    try:
        import concourse  # noqa: F401
    except ImportError:
        import sys

        for p in ("/opt/trn_rl_repo", "/root/.axon_site/_ro/trn_rl_repo"):
            if p not in sys.path:
                sys.path.insert(0, p)


N_CORES = 8
B, D, L, K, NUM = 4, 64, 4096, 200, 32
NW = 34  # x windows of 128 per pair tile (covers t+j up to 4351)
NT = 32  # output time blocks of 128
NK = 33  # khat product blocks (u = t + 100 spans [0, 4224))
PAIRS = 8  # row-pairs per core (16 rows)


DEFAULT_CFG = {
    "fill": "A2",         # A2: like A but qt in two half DMAs
    "kmul_split": (),     # pairs whose kmul is window-chunked
    "qmul_split": tuple(range(8)),  # pairs whose qmul is split in halves
    "tail_break": True,   # emit qreduce(LAST) immediately at slot LAST+1
    "last_tree_dve": False,
    "last_tail_dve": True,
    "drain_chunks": 4,    # conv/drain chunks per pair (divisor of 32)
    "warmup": 0,          # dummy PE matmuls after wq load (p-state pre-ramp)
    "xsplit": 18,         # first-chunk windows of each xt load
    "xbufs": 3,
    "bufs3": False,       # peq/pq/pk/tree pools with 3 buffers
    "kmul_pos": 0,        # 0: before conv; 1: after conv half 0
    "tail_dve_pairs": (), # extra pairs whose tree tail runs on DVE
    "kmul_bcast": False,  # single-instruction kmul via broadcast keys AP
    "l1_split": False,    # Pool L1 as two half-adds (finer overlap)
    "l4_dve": False,      # move tree L4 from Pool to DVE
}


def build_module(cfg=None):
    """Build + compile the per-core Bass module (identical SPMD program)."""
    cfg = {**DEFAULT_CFG, **(cfg or {})}
    _ensure_paths()
    from contextlib import ExitStack

    import concourse.bacc as bacc
    import concourse.mybir as mybir
    import concourse.tile as tile

    F16 = mybir.dt.float16
    F32 = mybir.dt.float32
    X = mybir.AxisListType.X

    nc = bacc.Bacc(
        "TRN2", target_bir_lowering=False, debug=False, num_devices=N_CORES
    )

    xf_d = nc.dram_tensor("xf", [PAIRS, 128, NW, 128], F16, kind="ExternalInput").ap()
    # packed consts: [3*128 Toeplitz cols | 2 selector cols]
    wq_d = nc.dram_tensor("wq", [128, 3 * 128 + 2], F16, kind="ExternalInput").ap()
    qt_d = nc.dram_tensor("qt", [128, NT, 128], F16, kind="ExternalInput").ap()
    kt_d = nc.dram_tensor("kt", [128, NK, D], F16, kind="ExternalInput").ap()
    out_d = nc.dram_tensor(
        "out", [PAIRS, 128, NT + NK, 2], F16, kind="ExternalOutput"
    ).ap()

    with tile.TileContext(nc) as tc, ExitStack() as ctx:
        wp = ctx.enter_context(tc.tile_pool(name="const", bufs=1))
        xp = ctx.enter_context(tc.tile_pool(name="x", bufs=cfg["xbufs"]))
        pp = ctx.enter_context(tc.tile_pool(name="psum", bufs=2, space="PSUM"))
        op_ps = ctx.enter_context(tc.tile_pool(name="opsum", bufs=2, space="PSUM"))
        nb = 3 if cfg["bufs3"] else 2
        cp = ctx.enter_context(tc.tile_pool(name="peq", bufs=nb))
        qp = ctx.enter_context(tc.tile_pool(name="prodq", bufs=nb))
        kp = ctx.enter_context(tc.tile_pool(name="prodk", bufs=nb))
        tp = ctx.enter_context(tc.tile_pool(name="tree", bufs=nb))
        oq = ctx.enter_context(tc.tile_pool(name="outq", bufs=2))

        # consts: Toeplitz weights + selector in one DMA (small, gates conv)
        wq_t = wp.tile([128, 3 * 128 + 2], F16, tag="wq")
        nc.sync.dma_start(wq_t[:], wq_d[:])
        wts = [wq_t[:, 128 * s : 128 * (s + 1)] for s in range(3)]
        sel_t = wq_t[:, 384:386]

        if cfg["warmup"]:
            # dummy matmuls on the weights tile: keeps PE busy through its
            # p-state ramp while the first x tiles stream in
            wu = op_ps.tile([128, 128], F32, tag="wu")
            for _ in range(cfg["warmup"]):
                nc.tensor.matmul(wu[:], wts[0], wts[1], start=True, stop=True)

        xts = {}

        XS = cfg["xsplit"]

        def load_x(pr):
            """Two-chunk load so the conv can start on the first half."""
            if pr >= PAIRS:
                return
            xt = xp.tile([128, NW, 128], F16, tag="xt", name=f"xt_{pr}")
            nc.sync.dma_start(xt[:, 0:XS, :], xf_d[pr, :, 0:XS, :])
            nc.sync.dma_start(xt[:, XS:NW, :], xf_d[pr, :, XS:NW, :])
            xts[pr] = xt

        kt_t = wp.tile([128, NK, D], F16, tag="kt")
        qt_t = wp.tile([128, NT, 128], F16, tag="qt")
        if cfg["fill"] == "A":
            load_x(0)
            nc.sync.dma_start(kt_t[:], kt_d[:])
            load_x(1)
            nc.sync.dma_start(qt_t[:], qt_d[:])
        elif cfg["fill"] == "A2":
            load_x(0)
            nc.sync.dma_start(kt_t[:], kt_d[:])
            load_x(1)
            nc.sync.dma_start(qt_t[:, 0:16, :], qt_d[:, 0:16, :])
            nc.sync.dma_start(qt_t[:, 16:NT, :], qt_d[:, 16:NT, :])
        elif cfg["fill"] == "A3":
            load_x(0)
            nc.sync.dma_start(kt_t[:], kt_d[:])
            xt1 = xp.tile([128, NW, 128], F16, tag="xt", name="xt_1")
            nc.sync.dma_start(xt1[:, 0:XS, :], xf_d[1, :, 0:XS, :])
            nc.sync.dma_start(qt_t[:, 0:16, :], qt_d[:, 0:16, :])
            nc.sync.dma_start(xt1[:, XS:NW, :], xf_d[1, :, XS:NW, :])
            nc.sync.dma_start(qt_t[:, 16:NT, :], qt_d[:, 16:NT, :])
            xts[1] = xt1
        elif cfg["fill"] == "C":
            xt0 = xp.tile([128, NW, 128], F16, tag="xt", name="xt_0")
            xt1 = xp.tile([128, NW, 128], F16, tag="xt", name="xt_1")
            nc.sync.dma_start(xt0[:, 0:XS, :], xf_d[0, :, 0:XS, :])
            nc.sync.dma_start(kt_t[:, 0:XS, :], kt_d[:, 0:XS, :])
            nc.sync.dma_start(xt0[:, XS:NW, :], xf_d[0, :, XS:NW, :])
            nc.sync.dma_start(kt_t[:, XS:NK, :], kt_d[:, XS:NK, :])
            nc.sync.dma_start(xt1[:, 0:XS, :], xf_d[1, :, 0:XS, :])
            nc.sync.dma_start(qt_t[:, 0:16, :], qt_d[:, 0:16, :])
            nc.sync.dma_start(xt1[:, XS:NW, :], xf_d[1, :, XS:NW, :])
            nc.sync.dma_start(qt_t[:, 16:NT, :], qt_d[:, 16:NT, :])
            xts[0] = xt0
            xts[1] = xt1
        else:
            xt0 = xp.tile([128, NW, 128], F16, tag="xt", name="xt_0")
            xt1 = xp.tile([128, NW, 128], F16, tag="xt", name="xt_1")
            nc.sync.dma_start(xt0[:, 0:18, :], xf_d[0, :, 0:18, :])
            nc.sync.dma_start(kt_t[:], kt_d[:])
            nc.sync.dma_start(xt0[:, 18:NW, :], xf_d[0, :, 18:NW, :])
            nc.sync.dma_start(xt1[:, 0:18, :], xf_d[1, :, 0:18, :])
            nc.sync.dma_start(qt_t[:, 0:16, :], qt_d[:, 0:16, :])
            nc.sync.dma_start(xt1[:, 18:NW, :], xf_d[1, :, 18:NW, :])
            nc.sync.dma_start(qt_t[:, 16:NT, :], qt_d[:, 16:NT, :])
            xts[0] = xt0
            xts[1] = xt1

        peq_t, pq_t, pk_t, t1_t, t3_t, out_t = {}, {}, {}, {}, {}, {}

        NCH = cfg["drain_chunks"]
        CW = NT // NCH

        def conv_chunk(pr, half):
            """Emit conv+drain for half in {0,1} as NCH/2 chunks."""
            for ch in range(half * NCH // 2, (half + 1) * NCH // 2):
                conv_chunk1(pr, ch)

        def conv_chunk1(pr, ch):
            if not (0 <= pr < PAIRS) or pr not in xts:
                return
            xt = xts[pr]
            if ch == 0:
                peq_t[pr] = cp.tile([128, NT, 128], F16, tag="peq", name=f"peq_{pr}")
            ps = pp.tile([128, CW, 128], F32, tag="ps", name=f"ps_{pr}_{ch}")
            for t8 in range(CW):
                tau = ch * CW + t8
                # W_2 is zero for output cols < 57: emit it narrow, mid-group
                nc.tensor.matmul(
                    ps[:, t8, :], xt[:, tau, :], wts[0], start=True, stop=False
                )
                nc.tensor.matmul(
                    ps[:, t8, 57:128],
                    xt[:, tau + 2, :],
                    wq_t[:, 313:384],
                    start=False,
                    stop=False,
                    skip_group_check=True,
                )
                nc.tensor.matmul(
                    ps[:, t8, :], xt[:, tau + 1, :], wts[1], start=False, stop=True
                )
            nc.scalar.copy(peq_t[pr][:, ch * CW : (ch + 1) * CW, :], ps[:])

        def kmul(pr):
            if not (0 <= pr < PAIRS):
                return
            xt = xts[pr]
            pk = kp.tile([128, NK, 2, D], F16, tag="pk", name=f"pk_{pr}")
            if pr in cfg["kmul_split"]:
                # window-chunked so DVE can start on the first x/keys chunks
                for r in range(2):
                    nc.vector.tensor_mul(
                        pk[:, 0:18, r, :],
                        xt[:, 0:18, D * r : D * (r + 1)],
                        kt_t[:, 0:18, :],
                    )
                for r in range(2):
                    nc.vector.tensor_mul(
                        pk[:, 18:NK, r, :],
                        xt[:, 18:NK, D * r : D * (r + 1)],
                        kt_t[:, 18:NK, :],
                    )
            elif cfg["kmul_bcast"]:
                xv = xt[:, 0:NK, :].rearrange("p n (r d) -> p n r d", r=2)
                ktb = kt_t[:].unsqueeze(2).broadcast_to((128, NK, 2, D))
                nc.vector.tensor_mul(pk[:], xv, ktb)
            else:
                nc.vector.tensor_mul(pk[:, :, 0, :], xt[:, 0:NK, 0:D], kt_t[:])
                nc.vector.tensor_mul(pk[:, :, 1, :], xt[:, 0:NK, D:128], kt_t[:])
            pk_t[pr] = pk

        def qmul(pr):
            if not (0 <= pr < PAIRS):
                return
            peq = peq_t.pop(pr)
            pq = qp.tile([128, NT, 128], F16, tag="pq", name=f"pq_{pr}")
            if pr in cfg["qmul_split"]:
                # halves: starts earlier at fill, drains faster at the tail
                h = NT // 2
                nc.vector.tensor_mul(pq[:, 0:h, :], peq[:, 0:h, :], qt_t[:, 0:h, :])
                nc.vector.tensor_mul(
                    pq[:, h:NT, :], peq[:, h:NT, :], qt_t[:, h:NT, :]
                )
            else:
                nc.vector.tensor_mul(pq[:], peq[:], qt_t[:])
            pq_t[pr] = pq

        def qreduce(pr):
            """PE d-reduce for pair pr, then drain + DMA out."""
            if not (0 <= pr < PAIRS):
                return
            pq = pq_t.pop(pr)
            po = op_ps.tile([128, NT, 2], F32, tag="po", name=f"po_{pr}")
            for t in range(NT):
                nc.tensor.matmul(
                    po[:, t, :], pq[:, t, :], sel_t, start=True, stop=True
                )
            out_s = oq.tile([128, NT + NK, 2], F16, tag="out", name=f"out_{pr}")
            nc.scalar.copy(out_s[:, 0:NT, :], po[:])
            out_t[pr] = out_s

        def tree_hi(pr, on_dve=False):
            """L1 (Pool unless on_dve) producing t1."""
            if not (0 <= pr < PAIRS):
                return
            pk = pk_t.pop(pr)
            eng = nc.vector if on_dve else nc.gpsimd
            t1 = tp.tile([128, NK, 2, 32], F16, tag="t1", name=f"t1_{pr}")
            if cfg["l1_split"] and not on_dve:
                eng.tensor_add(
                    t1[:, 0:17], pk[:, 0:17, :, 0:32], pk[:, 0:17, :, 32:64]
                )
                eng.tensor_add(
                    t1[:, 17:NK], pk[:, 17:NK, :, 0:32], pk[:, 17:NK, :, 32:64]
                )
            else:
                eng.tensor_add(t1[:], pk[:, :, :, 0:32], pk[:, :, :, 32:64])
            t1_t[pr] = t1

        def tree_mid(pr):
            """L2+L3 on DVE producing t3."""
            if not (0 <= pr < PAIRS):
                return
            t1 = t1_t.pop(pr)
            t2 = tp.tile([128, NK, 2, 16], F16, tag="t2", name=f"t2_{pr}")
            nc.vector.tensor_add(t2[:], t1[:, :, :, 0:16], t1[:, :, :, 16:32])
            t3 = tp.tile([128, NK, 2, 8], F16, tag="t3", name=f"t3_{pr}")
            nc.vector.tensor_add(t3[:], t2[:, :, :, 0:8], t2[:, :, :, 8:16])
            t3_t[pr] = t3

        def tree_tail(pr, on_dve=False):
            """L4-L6 (Pool unless on_dve) -> ko DMA."""
            if not (0 <= pr < PAIRS):
                return
            t3 = t3_t.pop(pr)
            eng = nc.vector if on_dve else nc.gpsimd
            e4 = nc.vector if (cfg["l4_dve"] and not on_dve) else eng
            t4 = tp.tile([128, NK, 2, 4], F16, tag="t4", name=f"t4_{pr}")
            e4.tensor_add(t4[:], t3[:, :, :, 0:4], t3[:, :, :, 4:8])
            t5 = tp.tile([128, NK, 2, 2], F16, tag="t5", name=f"t5_{pr}")
            eng.tensor_add(t5[:], t4[:, :, :, 0:2], t4[:, :, :, 2:4])
            out_s = out_t.pop(pr)
            eng.tensor_add(
                out_s[:, NT : NT + NK, :], t5[:, :, :, 0], t5[:, :, :, 1]
            )
            nc.sync.dma_start(out_d[pr], out_s[:])

        LAST = PAIRS - 1
        for i in range(PAIRS + 2):
            load_x(i + 2)
            if cfg["kmul_pos"] == 0:
                kmul(i)
            conv_chunk(i, 0)
            if cfg["kmul_pos"] == 1:
                kmul(i)
            qreduce(i - 2)
            conv_chunk(i, 1)
            qmul(i - 1)
            if i - 1 == LAST and cfg["last_tree_dve"]:
                # last pair's tree entirely on DVE (after qmul so the qo
                # output chain starts first) to shorten the tail
                tree_hi(LAST, on_dve=True)
                tree_mid(LAST)
                tree_tail(LAST, on_dve=True)
            else:
                tree_hi(i - 1)
                tree_mid(i - 1)
            if i - 2 != LAST or not cfg["last_tree_dve"]:
                tree_tail(
                    i - 2,
                    on_dve=(i - 2 == LAST and cfg["last_tail_dve"])
                    or (i - 2) in cfg["tail_dve_pairs"],
                )
            if i - 1 == LAST and cfg["tail_break"]:
                qreduce(LAST)
                tree_tail(LAST, on_dve=cfg["last_tail_dve"])
                break

    nc.compile()
    return nc


def _get_module():
    if "nc" not in _CACHE:
        _CACHE["nc"] = build_module()
    return _CACHE["nc"]


def make_in_maps(queries, keys, noise, conv_weight, num):
    """Host-side shard + re-layout (all cheap numpy ops)."""
    num = int(np.asarray(num))
    queries = np.asarray(queries, dtype=np.float32)
    keys = np.asarray(keys, dtype=np.float32)
    noise = np.asarray(noise, dtype=np.float32)
    w = np.asarray(conv_weight, dtype=np.float32)[0, 0, :]
    scale = 1.0 / math.sqrt(num * D)

    # Toeplitz weights (scale folded in): W_s[p, m] = w[p + 128s - m] * scale
    p = np.arange(128)[:, None]
    m = np.arange(128)[None, :]
    Wq = np.zeros((3, 128, 128), np.float32)
    for s in range(3):
        j = p + 128 * s - m
        mask = (j >= 0) & (j < K)
        Wq[s][mask] = w[j[mask]] * scale
    Wq16 = Wq.astype(np.float16)

    # row selector: sel[rd, r'] = (rd // 64 == r')
    sel = np.zeros((128, 2), np.float16)
    sel[0:D, 0] = 1.0
    sel[D:128, 1] = 1.0
    # packed [128, 3*128+2]: Toeplitz cols s-major then selector
    wq_pack = np.concatenate(
        [Wq16.transpose(1, 0, 2).reshape(128, 3 * 128), sel], axis=1
    )

    # xf[c][pair][p, n, (r,d)] = noise[16c + 2*pair + r, d, 128n + p]
    xf = (
        noise[:, :, : NW * 128]
        .reshape(N_CORES, PAIRS, 2, D, NW, 128)
        .transpose(0, 1, 5, 4, 2, 3)
        .reshape(N_CORES, PAIRS, 128, NW, 128)
        .astype(np.float16)
    )
    # qt[b][(r,d), tau, t'] = queries[b, d, 128 tau + t']  (replicated over r)
    qt_half = queries.reshape(B, D, NT, 128).astype(np.float16)
    qt = np.concatenate([qt_half, qt_half], axis=1)  # (B, 128, NT, 128)
    # kt[b][p, n, d] = keys[b, d, 128n + p - 100] * scale (zero out of range)
    kpad = np.zeros((B, D, NK * 128), np.float32)
    kpad[:, :, K // 2 : K // 2 + L] = keys * scale
    kt = kpad.reshape(B, D, NK, 128).transpose(0, 3, 2, 1).astype(np.float16)

    in_maps = []
    for c in range(N_CORES):
        b = c // 2
        in_maps.append(
            {
                "xf": np.ascontiguousarray(xf[c]),
                "wq": wq_pack,
                "qt": np.ascontiguousarray(qt[b]),
                "kt": np.ascontiguousarray(kt[b]),
            }
        )
    return in_maps


def assemble_outputs(results):
    qhat = np.empty((B * NUM, L), np.float32)
    khat = np.empty((B * NUM, L), np.float32)
    for c in range(N_CORES):
        out = results[c]["out"].astype(np.float32)  # (PAIRS, 128, NT+NK, 2)
        qarr = out[:, :, 0:NT, :].transpose(0, 3, 2, 1).reshape(16, NT * 128)
        karr = (
            out[:, :, NT : NT + NK, :].transpose(0, 3, 2, 1).reshape(16, NK * 128)
        )
        qhat[16 * c : 16 * (c + 1)] = qarr
        khat[16 * c : 16 * (c + 1)] = karr[:, K // 2 : K // 2 + L]
    return (
        qhat.reshape(B, NUM, L),
        khat.reshape(B, NUM, L),
    )


def kernel(queries, keys, noise, conv_weight, num):
    _ensure_paths()
    from concourse import bass_utils

    in_maps = make_in_maps(queries, keys, noise, conv_weight, num)
    nc = _get_module()
    res = bass_utils.run_bass_kernel_spmd(nc, in_maps, core_ids=list(range(N_CORES)))
    return assemble_outputs(res.results)
